# revision 13
# baseline (speedup 1.0000x reference)
"""Distributed Trainium2 Bass kernel for multi-head attention.

Problem: x[2,2048,2048] @ qkv_w[2048,6144] -> rope(q,k) -> softmax(qk^T/sqrt(d)) @ v
         -> concat heads -> @ out_w[2048,2048].

Sharding (8 cores): core i handles batch b = i//4 and head group g = i%4
(heads 4g..4g+3).  out_w is ROW-sharded by head group: each core computes a
partial output  out_g = attn_g @ out_w[512g:512(g+1), :]  over the FULL 2048
output columns, written as bf16; the host sums the four per-group partials
per batch.  No collectives at all -- cores are fully independent, so core
launch skew and collective latency cannot stall the PE (the previous
AllGather-based version lost ~50us/run to gather waits).

All inputs cast to bf16 on the host.  Each core:
  1. qT,kT = (Wqk_g^T x_b^T) with rope applied          [8 x [128, 2048]]
     (K half before Q half per token chunk; chunk-0 x/wqkK tiles are
     interleaved k-major across HWDGE/SWDGE queues for arrival pacing; all
     later loads are single wide DMAs -- the shared HWDGE descriptor
     generator (~630ns/DMA) limits issue rate, not bandwidth).
  2. v     = x_b @ Wv_g  (natural layout)               [16 x [128, 512]]
  3. per head h, query chunk jq (512 queries), double-iter over key pairs:
       two score matmuls land side by side in ONE 2-bank PSUM tile
       [128,1024]; a single 1024-wide exp on the scalar engine halves the
       per-op ACT overhead (scalar would otherwise pace the chunk: 16x720ns
       > PE's 16x639ns); out^T += v^T P accumulated in PSUM over 16 tiles.
       Denominator: P tiles summed via two chains (DVE 9 ops / GpSimd 5 ops
       -- the DVE alone cannot absorb accumulation + the finish chain),
       merged on DVE, then gpsimd.partition_all_reduce over keys and a
       single-pass reciprocal_approx_fast on the DVE (the exact DVE
       reciprocal is ~3.3us for [1,512] and serialized the accum chain ->
       p-slot starvation -> exp stalls -> PE stalls, every head).
  4. out-projection of chunk j-1 (64 matmuls: 4 token tiles x 4 col tiles
     accumulated over the 4 local head tiles) interleaved one matmul per
     exp inside chunk j's attention loops; chunk 0 interleaves the deferred
     last-token-chunk Q projection instead.  Output tiles DMA directly from
     the drain thunks on the sync queue (nothing else uses it).
Engine roles in the attention phase: scalar=exp only; DVE=partial P sums,
merge, reciprocal, normalize, PSUM->SBUF drains; GpSimd=partial P sums +
partition_all_reduce; sync=output DMA.  ~56 warm-up matmuls on a zeroed
tile bridge the initial DMA wait so the PE HAM clock-gate (cold = 1.2GHz
for the first ~3.4us of activity) is released before real work arrives.
Host: slices/transposes/casts inputs per core, sums partial outputs.
"""

import numpy as np
import ml_dtypes

from concourse import bacc, bass_isa, mybir, tile
from concourse.bass_utils import run_bass_kernel_spmd

B, N, HID = 2, 2048, 2048
H, D = 16, 128
G = 4              # head groups (tensor parallel within a batch)
HG = H // G        # heads per group
QK_COLS = HG * D   # 512
NT = N // 128      # 16 token tiles
KT = HID // 128    # 16 hidden tiles
TC = 512           # free-dim chunk
NTC = N // TC      # 4
OPC = 512          # output column tile
NOC = HID // OPC   # 4

F32 = mybir.dt.float32
BF16 = mybir.dt.bfloat16
SCALE = float(1.0 / np.sqrt(D))
SWAP_MASK = [p ^ 1 for p in range(32)]  # adjacent-pair swap, uniform per 32-lane group
WARMUP_MMS = 56

_NC = None
LAST_RESULT = None


def _build():
    nc = bacc.Bacc("TRN2", target_bir_lowering=False, debug=False, num_devices=8)

    xT = nc.dram_tensor("xT", [HID, N], BF16, kind="ExternalInput")
    wqk = nc.dram_tensor("wqk", [HID, 2 * QK_COLS], BF16, kind="ExternalInput")
    wv = nc.dram_tensor("wv", [HID, QK_COLS], BF16, kind="ExternalInput")
    wo = nc.dram_tensor("wo", [QK_COLS, HID], BF16, kind="ExternalInput")
    cosT = nc.dram_tensor("cosT", [D, N], BF16, kind="ExternalInput")
    sinT = nc.dram_tensor("sinT", [D, N], BF16, kind="ExternalInput")
    out = nc.dram_tensor("out", [N, HID], BF16, kind="ExternalOutput")

    with tile.TileContext(nc) as tc:
        with tc.tile_pool(name="pqkv", bufs=1) as pqkv:
            qkT = [pqkv.tile([128, N], BF16, name=f"qkT{m}", tag=f"qkT{m}") for m in range(2 * HG)]
            v_sb = [pqkv.tile([128, QK_COLS], BF16, name=f"v{t}", tag=f"v{t}") for t in range(NT)]
            # persistent: the last token chunk's Q-projection is deferred into
            # attention chunk 0 as PE filler; its rope output is first read by
            # chunk-3 attention.
            wqkQ_sb = pqkv.tile([128, KT * QK_COLS], BF16, name="wqkQ", tag="wqkQ")
            xt3 = pqkv.tile([128, KT * TC], BF16, name="xt3", tag="xt3")
            cos_sb = pqkv.tile([D, N], BF16, name="cos_sb", tag="cos")
            sin_sb = pqkv.tile([D, N], BF16, name="sin_sb", tag="sin")

            # ---- stages 1+2: q,k (transposed, roped) and v (natural) ----
            with (
                tc.tile_pool(name="s1w", bufs=1) as s1w,
                tc.tile_pool(name="s1x", bufs=1) as s1x,
                tc.tile_pool(name="s1t", bufs=3) as s1t,
                tc.tile_pool(name="psqk", bufs=8, space="PSUM") as psqk,
            ):
                wqkK_sb = [
                    s1w.tile([128, QK_COLS], BF16, name=f"wqkK{k}", tag=f"wqkK{k}")
                    for k in range(KT)
                ]
                wv_sb = s1w.tile([128, KT * QK_COLS], BF16, name="wv_sb", tag="wv_sb")
                xt12 = [
                    s1w.tile([128, KT * TC], BF16, name=f"xt12_{c}", tag=f"xt12_{c}")
                    for c in range(2)
                ]
                scratch = s1w.tile([128, 128], BF16, name="scratch", tag="scratch")

                # PE warm-up: ~56 back-to-back matmuls on a zeroed tile keep
                # the PE busy through the initial DMA wait so the HAM clock
                # gate opens (K=8/8) before the first real matmul.
                nc.vector.memset(scratch[:], 0.0)
                psw = [psqk.tile([128, 128], F32, name="psw", tag="psqk") for _ in range(2)]
                for i in range(WARMUP_MMS):
                    nc.tensor.matmul(
                        psw[i % 2][:, 0:128], scratch[:], scratch[:],
                        start=True, stop=True,
                    )

                # chunk-0 x/wqkK tiles: k-major interleave of (xt[k], wqkK[k])
                # pairs so tile pair k lands before the k-th matmul.  HWDGE
                # (sync+scalar) takes odds, SWDGE (gpsimd) takes even pairs.
                xt0 = [
                    s1x.tile([128, TC], BF16, name=f"xt{k}", tag=f"xt{k}")
                    for k in range(KT)
                ]
                for k in range(KT):
                    if k == 0:
                        nc.sync.dma_start(wqkK_sb[k][:], wqk[k * 128 : (k + 1) * 128, QK_COLS:])
                        nc.scalar.dma_start(xt0[k][:], xT[k * 128 : (k + 1) * 128, 0:TC])
                    elif k % 2 == 0:
                        nc.gpsimd.dma_start(xt0[k][:], xT[k * 128 : (k + 1) * 128, 0:TC])
                        nc.gpsimd.dma_start(wqkK_sb[k][:], wqk[k * 128 : (k + 1) * 128, QK_COLS:])
                    else:
                        nc.scalar.dma_start(xt0[k][:], xT[k * 128 : (k + 1) * 128, 0:TC])
                        nc.sync.dma_start(wqkK_sb[k][:], wqk[k * 128 : (k + 1) * 128, QK_COLS:])

                # wide DMAs for everything else (one descriptor each; the
                # fabric spreads packets over all 16 DMA engines).  Layout of
                # the k-blocked tiles: column block k holds hidden-tile k.
                # wqkQ in 4 paced quarters on the scalar queue (behind the
                # xt0 odds): quarter q lands just before the Q-half consumes
                # hidden-tiles 4q..4q+3 (a single 2MB DMA finished ~12us
                # after the K half was ready)
                for q in range(4):
                    nc.scalar.dma_start(
                        wqkQ_sb[:, q * 4 * QK_COLS : (q + 1) * 4 * QK_COLS].rearrange(
                            "p (k c) -> p k c", k=4
                        ),
                        wqk[q * 4 * 128 : (q + 1) * 4 * 128, :QK_COLS].rearrange(
                            "(k p) c -> p k c", k=4
                        ),
                    )
                nc.sync.dma_start(cos_sb[:], cosT[:])
                nc.sync.dma_start(sin_sb[:], sinT[:])
                nc.sync.dma_start(
                    xt12[0][:].rearrange("p (k c) -> p k c", k=KT),
                    xT[:, TC : 2 * TC].rearrange("(k p) c -> p k c", k=KT),
                )
                for hw in range(2):  # wv in halves so chunk-0 V paces
                    nc.gpsimd.dma_start(
                        wv_sb[:, hw * 8 * QK_COLS : (hw + 1) * 8 * QK_COLS].rearrange(
                            "p (k c) -> p k c", k=8
                        ),
                        wv[hw * 8 * 128 : (hw + 1) * 8 * 128, :].rearrange(
                            "(k p) c -> p k c", k=8
                        ),
                    )
                nc.sync.dma_start(
                    xt12[1][:].rearrange("p (k c) -> p k c", k=KT),
                    xT[:, 2 * TC : 3 * TC].rearrange("(k p) c -> p k c", k=KT),
                )
                nc.scalar.dma_start(
                    xt3[:].rearrange("p (k c) -> p k c", k=KT),
                    xT[:, 3 * TC : 4 * TC].rearrange("(k p) c -> p k c", k=KT),
                )

                def xt_slice(tcn, k):
                    if tcn == 0:
                        return xt0[k][:]
                    if tcn == 3:
                        return xt3[:, k * TC : (k + 1) * TC]
                    return xt12[tcn - 1][:, k * TC : (k + 1) * TC]

                for tcn in range(NTC):
                    tsl = slice(tcn * TC, (tcn + 1) * TC)
                    for half in (1, 0):  # K half first (earlier attention readiness)
                        if half == 0 and tcn == NTC - 1:
                            continue  # last chunk's Q half deferred to stage 3
                        psums = [
                            psqk.tile([128, TC], F32, name="psqk", tag="psqk")
                            for _ in range(4)
                        ]
                        if tcn == 0 and half == 1:
                            # consume HWDGE-delivered odd tiles before the
                            # slower SWDGE evens (accumulation order is free)
                            korder = [0] + list(range(1, KT, 2)) + list(range(2, KT, 2))
                        else:
                            korder = list(range(KT))
                        for ki, k in enumerate(korder):
                            for mi in range(4):
                                if half == 1:
                                    wh = wqkK_sb[k][:, mi * 128 : (mi + 1) * 128]
                                else:
                                    wh = wqkQ_sb[:, k * QK_COLS + mi * 128 : k * QK_COLS + (mi + 1) * 128]
                                nc.tensor.matmul(
                                    psums[mi][:],
                                    wh,
                                    xt_slice(tcn, k),
                                    start=(ki == 0),
                                    stop=(ki == KT - 1),
                                )
                        for mi in range(4):
                            m = half * 4 + mi
                            qsb = s1t.tile([128, TC], BF16, tag="qsb")
                            nc.scalar.activation(qsb[:], psums[mi][:], mybir.ActivationFunctionType.Copy)
                            shuf = s1t.tile([128, TC], BF16, tag="shuf")
                            nc.vector.stream_shuffle(shuf[:], qsb[:], SWAP_MASK)
                            t1 = s1t.tile([128, TC], F32, tag="t1")
                            nc.vector.tensor_tensor(
                                t1[:], qsb[:], cos_sb[:, tsl], mybir.AluOpType.mult
                            )
                            t2 = s1t.tile([128, TC], F32, tag="t2")
                            nc.vector.tensor_tensor(
                                t2[:], shuf[:], sin_sb[:, tsl], mybir.AluOpType.mult
                            )
                            nc.vector.tensor_tensor(
                                qkT[m][:, tsl], t1[:], t2[:], mybir.AluOpType.add
                            )
                    for mtl in range(4):
                        mt = tcn * 4 + mtl
                        pv = psqk.tile([128, QK_COLS], F32, name="psv", tag="psqk")
                        for k in range(KT):
                            nc.tensor.matmul(
                                pv[:],
                                xt_slice(tcn, k)[:, mtl * 128 : (mtl + 1) * 128],
                                wv_sb[:, k * QK_COLS : (k + 1) * QK_COLS],
                                start=(k == 0),
                                stop=(k == KT - 1),
                            )
                        if tcn < NTC - 1:
                            # scalar is idle here; keep the DVE free for rope
                            nc.scalar.activation(v_sb[mt][:], pv[:], mybir.ActivationFunctionType.Copy)
                        else:
                            # last chunk: scalar must reach the attention exps
                            # without these queued ahead of them
                            nc.vector.tensor_copy(v_sb[mt][:], pv[:])

            # ---- stages 3-5: attention; out projection of chunk j-1
            # interleaved into the attention loops of chunk j ----
            with (
                tc.tile_pool(name="s3p", bufs=8) as s3p,
                tc.tile_pool(name="s3r", bufs=2) as s3r,
                tc.tile_pool(name="s3s", bufs=4) as s3s,
                tc.tile_pool(name="s3d", bufs=2) as s3d,
                tc.tile_pool(name="s3a", bufs=2) as s3a,
                tc.tile_pool(name="s5w", bufs=1) as s5w,
                tc.tile_pool(name="s5o", bufs=4) as s5o,
                tc.tile_pool(name="pss", bufs=2, space="PSUM") as pss,
                tc.tile_pool(name="pso", bufs=2, space="PSUM") as pso,
                tc.tile_pool(name="psf", bufs=2, space="PSUM") as psf,
            ):
                wo_sb = [s5w.tile([128, HID], BF16, name=f"wo{h}", tag=f"wo{h}") for h in range(HG)]
                for h in range(HG):
                    nc.sync.dma_start(wo_sb[h][:], wo[h * 128 : (h + 1) * 128, :])

                pob_live = {}  # (jq, h) -> normalized bf16 attention tile
                pending_finish = []  # deferred recip+norm emissions; flushed
                # a few iterations into the NEXT head so the reciprocal's
                # wait on the gpsimd all_reduce never head-of-line blocks
                # drains already queued on the DVE FIFO

                def attention_head(jq, h, filler=None):
                    """filler: list of thunks (PE matmul emissions) doled out
                    between the scores and AV matmuls so the PE has work
                    while the scalar engine computes exp."""
                    q0 = jq * TC
                    qsl = slice(q0, q0 + TC)
                    po = pso.tile([128, TC], F32, name="pso", tag="pso")
                    ps_w = s3s.tile([128, 2 * TC], BF16, name="psw", tag="psw")
                    plist = []
                    filler = list(filler) if filler else []
                    for ib in range(NT // 2):
                        ik0, ik1 = 2 * ib, 2 * ib + 1
                        psb = pss.tile([128, 2 * TC], F32, name="psb", tag="psb")
                        nc.tensor.matmul(
                            psb[:, :TC],
                            qkT[HG + h][:, ik0 * 128 : (ik0 + 1) * 128],
                            qkT[h][:, qsl],
                            start=True, stop=True,
                        )
                        nc.tensor.matmul(
                            psb[:, TC:],
                            qkT[HG + h][:, ik1 * 128 : (ik1 + 1) * 128],
                            qkT[h][:, qsl],
                            start=True, stop=True,
                        )
                        p = s3p.tile([128, 2 * TC], BF16, name="p", tag="p")
                        plist.append(p)
                        nc.scalar.activation(
                            p[:], psb[:], mybir.ActivationFunctionType.Exp, scale=SCALE
                        )
                        if filler:
                            filler.pop(0)()
                        nc.tensor.matmul(
                            po[:],
                            v_sb[ik0][:, h * 128 : (h + 1) * 128],
                            p[:, :TC],
                            start=(ib == 0), stop=False,
                        )
                        if filler:
                            filler.pop(0)()
                        nc.tensor.matmul(
                            po[:],
                            v_sb[ik1][:, h * 128 : (h + 1) * 128],
                            p[:, TC:],
                            start=False, stop=(ib == NT // 2 - 1),
                        )
                        # denominator partial sums: one 1024-wide DVE chain
                        if ib == 1:
                            nc.vector.tensor_tensor(ps_w[:], plist[0][:], plist[1][:], mybir.AluOpType.add)
                        elif ib > 1:
                            nc.vector.tensor_tensor(ps_w[:], ps_w[:], p[:], mybir.AluOpType.add)
                        if ib == 2 and pending_finish:
                            # previous head's all_reduce halves are done by
                            # now; its recip+norm emit here wait-free
                            for fn in pending_finish:
                                fn()
                            pending_finish.clear()
                    for f in filler:
                        f()
                    return po, ps_w

                HTC = TC // 2

                def attention_head_finish(jq, h, po, ps_w):
                    # fold + all_reduce in token-halves: the first half's
                    # denominator (tokens 0:256, which the first staggered
                    # out-projection groups read) is ready ~1.9us earlier
                    ps_f = s3s.tile([128, TC], BF16, name="psf2", tag="psf2")
                    den = s3d.tile([128, TC], F32, name="den", tag="den")
                    for a in (0, 1):
                        sl = slice(a * HTC, a * HTC + HTC)
                        slw = slice(TC + a * HTC, TC + a * HTC + HTC)
                        nc.vector.tensor_tensor(ps_f[:, sl], ps_w[:, sl], ps_w[:, slw], mybir.AluOpType.add)
                        nc.gpsimd.partition_all_reduce(den[:, sl], ps_f[:, sl], 128, bass_isa.ReduceOp.add)

                    def rest():
                        rden = s3d.tile([128, TC], F32, name="rden", tag="rden")
                        pob = s3a.tile([128, TC], BF16, name=f"pob{h}", tag=f"pob{h}")
                        for a in (0, 1):
                            sl = slice(a * HTC, a * HTC + HTC)
                            nc.vector.reciprocal_approx_fast(rden[:, sl], den[:, sl])
                            nc.vector.tensor_tensor(pob[:, sl], po[:, sl], rden[:, sl], mybir.AluOpType.mult)
                        pob_live[(jq, h)] = pob

                    pending_finish.append(rest)

                def outproj_thunks(jq):
                    """64 PE-matmul thunks for chunk jq's out-projection:
                    16 (token-tile, col-tile) groups, each accumulating over
                    the 4 local head tiles; the last position drains PSUM
                    (DVE, every 4th group on the otherwise-idle scalar) and
                    DMAs the output tile on the sync queue.  The first two
                    groups' h=3 matmuls are deferred ~6 slots so the filler
                    stream doesn't wait on the previous chunk's last-head
                    finish chain (fold -> partition_all_reduce 3.5us ->
                    reciprocal -> normalize, ~6us after its last exp)."""
                    thunks = []
                    for gi, (mql, ft) in enumerate(
                        [(m, f) for m in range(4) for f in range(NOC)]
                    ):
                        mq = jq * 4 + mql
                        holder = {}

                        def mk(h, mql=mql, ft=ft, mq=mq, gi=gi, holder=holder):
                            def thunk():
                                if h == 0:
                                    holder["pf"] = psf.tile([128, OPC], F32, name="psf", tag="psf")
                                pf = holder["pf"]
                                nc.tensor.matmul(
                                    pf[:],
                                    pob_live[(jq, h)][:, mql * 128 : (mql + 1) * 128],
                                    wo_sb[h][:, ft * OPC : (ft + 1) * OPC],
                                    start=(h == 0),
                                    stop=(h == HG - 1),
                                )
                                if h == HG - 1:
                                    ob = s5o.tile([128, OPC], BF16, name="ob", tag="ob")
                                    if gi % 4 == 3:
                                        nc.scalar.activation(
                                            ob[:], pf[:], mybir.ActivationFunctionType.Copy
                                        )
                                    else:
                                        nc.vector.tensor_copy(ob[:], pf[:])
                                    nc.sync.dma_start(
                                        out[mq * 128 : (mq + 1) * 128, ft * OPC : (ft + 1) * OPC],
                                        ob[:],
                                    )
                            return thunk

                        thunks.append([mk(h) for h in range(HG)])
                    # stagger: G0[h0..h2], G1[h0..h2], G0[h3], G1[h3], rest
                    order = thunks[0][:3] + thunks[1][:3] + [thunks[0][3], thunks[1][3]]
                    for g in thunks[2:]:
                        order.extend(g)
                    return order

                def deferred_q_thunks(mi):
                    """The last token chunk's Q projection for head-group
                    column mi: 16 matmuls into a borrowed psf bank, then the
                    PSUM->bf16 copy and rope, writing qkT[mi][:, 1536:2048]
                    (first read by chunk-3 attention)."""
                    tsl = slice((NTC - 1) * TC, NTC * TC)
                    pq = psf.tile([128, TC], F32, name="psq", tag="psf")

                    def mk(k):
                        def thunk():
                            nc.tensor.matmul(
                                pq[:],
                                wqkQ_sb[:, k * QK_COLS + mi * 128 : k * QK_COLS + (mi + 1) * 128],
                                xt3[:, k * TC : (k + 1) * TC],
                                start=(k == 0),
                                stop=(k == KT - 1),
                            )
                            if k == KT - 1:
                                qsb = s3r.tile([128, TC], BF16, tag="qsb")
                                nc.scalar.activation(
                                    qsb[:], pq[:], mybir.ActivationFunctionType.Copy
                                )
                                shuf = s3r.tile([128, TC], BF16, tag="shuf")
                                nc.vector.stream_shuffle(shuf[:], qsb[:], SWAP_MASK)
                                t1 = s3r.tile([128, TC], F32, tag="t1")
                                nc.vector.tensor_tensor(
                                    t1[:], qsb[:], cos_sb[:, tsl], mybir.AluOpType.mult
                                )
                                t2 = s3r.tile([128, TC], F32, tag="t2")
                                nc.vector.tensor_tensor(
                                    t2[:], shuf[:], sin_sb[:, tsl], mybir.AluOpType.mult
                                )
                                nc.vector.tensor_tensor(
                                    qkT[mi][:, tsl], t1[:], t2[:], mybir.AluOpType.add
                                )
                        return thunk

                    return [mk(k) for k in range(KT)]

                for jq in range(NTC):
                    if jq == 0:
                        # cross-assigned (head h builds Q for head h+1) so the
                        # filler never writes the qkT tile the current head's
                        # scores are reading
                        fillers = [t for h in range(HG) for t in deferred_q_thunks((h + 1) % HG)]
                    else:
                        fillers = outproj_thunks(jq - 1)
                    for h in range(HG):
                        po, ps_w = attention_head(
                            jq, h, fillers[h * 16 : (h + 1) * 16]
                        )
                        attention_head_finish(jq, h, po, ps_w)
                # tail: last chunk's projection, back-to-back
                for fn in pending_finish:
                    fn()
                pending_finish.clear()
                for t in outproj_thunks(NTC - 1):
                    t()

    nc.compile()
    return nc


def _get_nc():
    global _NC
    if _NC is None:
        _NC = _build()
    return _NC


def _prep_in_maps(x, rope, qkv_w, out_w):
    x = np.asarray(x, np.float32)
    rope = np.asarray(rope, np.float32)
    qkv_w = np.asarray(qkv_w, np.float32)
    out_w = np.asarray(out_w, np.float32)

    bf = ml_dtypes.bfloat16
    freqs = rope[:, 0, :]  # [N, D]
    cosT = np.ascontiguousarray(np.repeat(freqs[:, 0::2], 2, axis=1).T).astype(bf)
    sinT = np.repeat(freqs[:, 1::2], 2, axis=1).T.copy()
    sinT[0::2, :] *= -1.0  # rope sign folded in: rot[2i] = -q[2i+1]
    sinT = np.ascontiguousarray(sinT).astype(bf)

    qkv3 = qkv_w.reshape(HID, 3, H, D)
    xTs = [np.ascontiguousarray(x[b].T).astype(bf) for b in range(B)]
    in_maps = []
    for core in range(8):
        b, g = core // G, core % G
        hs = slice(g * HG, (g + 1) * HG)
        wq = qkv3[:, 0, hs, :].reshape(HID, QK_COLS)
        wk = qkv3[:, 1, hs, :].reshape(HID, QK_COLS)
        in_maps.append(
            dict(
                xT=xTs[b],
                wqk=np.ascontiguousarray(np.concatenate([wq, wk], axis=1)).astype(bf),
                wv=np.ascontiguousarray(qkv3[:, 2, hs, :].reshape(HID, QK_COLS)).astype(bf),
                wo=np.ascontiguousarray(out_w[g * QK_COLS : (g + 1) * QK_COLS, :]).astype(bf),
                cosT=cosT,
                sinT=sinT,
            )
        )
    return in_maps


def kernel(x, rope, qkv_w, out_w):
    global LAST_RESULT
    nc = _get_nc()
    in_maps = _prep_in_maps(x, rope, qkv_w, out_w)
    res = run_bass_kernel_spmd(nc, in_maps, core_ids=list(range(8)))
    LAST_RESULT = res
    outs = [np.asarray(r["out"], dtype=np.float32) for r in res.results]
    full = np.stack(
        [sum(outs[b * G + g] for g in range(G)) for b in range(B)]
    )
    return full.astype(np.float32)


# revision 23
# speedup vs baseline: 1.2082x; 1.2082x over previous
"""Distributed Trainium2 Bass kernel for multi-head attention.

Problem: x[2,2048,2048] @ qkv_w[2048,6144] -> rope(q,k) -> softmax(qk^T/sqrt(d)) @ v
         -> concat heads -> @ out_w[2048,2048].

Sharding (8 cores): core i handles batch b = i//4 and head group g = i%4
(heads 4g..4g+3).  out_w is ROW-sharded by head group: each core computes a
partial output  out_g = attn_g @ out_w[512g:512(g+1), :]  over the FULL 2048
output columns, written as bf16; the host sums the four per-group partials
per batch.  No collectives at all -- cores are fully independent, so core
launch skew and collective latency cannot stall the PE (the previous
AllGather-based version lost ~50us/run to gather waits).

All inputs cast to bf16 on the host.  Each core:
  1. qT,kT = (Wqk_g^T x_b^T) with rope applied          [8 x [128, 2048]]
     (K half before Q half per token chunk; chunk-0 x/wqkK tiles are
     interleaved k-major across HWDGE/SWDGE queues for arrival pacing; all
     later loads are single wide DMAs -- the shared HWDGE descriptor
     generator (~630ns/DMA) limits issue rate, not bandwidth).
  2. v     = x_b @ Wv_g  (natural layout)               [16 x [128, 512]]
  3. per head h, query chunk jq (512 queries), double-iter over key pairs:
       two score matmuls land side by side in ONE 2-bank PSUM tile
       [128,1024]; a single 1024-wide exp on the scalar engine halves the
       per-op ACT overhead (scalar would otherwise pace the chunk: 16x720ns
       > PE's 16x639ns); out^T += v^T P accumulated in PSUM over 16 tiles.
       Denominator: P tiles summed via two chains (DVE 9 ops / GpSimd 5 ops
       -- the DVE alone cannot absorb accumulation + the finish chain),
       merged on DVE, then gpsimd.partition_all_reduce over keys and a
       single-pass reciprocal_approx_fast on the DVE (the exact DVE
       reciprocal is ~3.3us for [1,512] and serialized the accum chain ->
       p-slot starvation -> exp stalls -> PE stalls, every head).
  4. out-projection of chunk j-1 (64 matmuls: 4 token tiles x 4 col tiles
     accumulated over the 4 local head tiles) interleaved one matmul per
     exp inside chunk j's attention loops; chunk 0 interleaves the deferred
     last-token-chunk Q projection instead.  Output tiles DMA directly from
     the drain thunks on the sync queue (nothing else uses it).
Engine roles in the attention phase: scalar=exp only; DVE=partial P sums,
merge, reciprocal, normalize, PSUM->SBUF drains; GpSimd=partial P sums +
partition_all_reduce; sync=output DMA.  ~56 warm-up matmuls on a zeroed
tile bridge the initial DMA wait so the PE HAM clock-gate (cold = 1.2GHz
for the first ~3.4us of activity) is released before real work arrives.
Host: slices/transposes/casts inputs per core, sums partial outputs.
"""

import numpy as np
import ml_dtypes

from concourse import bacc, bass_isa, mybir, tile
from concourse.bass_utils import run_bass_kernel_spmd

B, N, HID = 2, 2048, 2048
H, D = 16, 128
G = 4              # head groups (tensor parallel within a batch)
HG = H // G        # heads per group
QK_COLS = HG * D   # 512
NT = N // 128      # 16 token tiles
KT = HID // 128    # 16 hidden tiles
TC = 512           # free-dim chunk
NTC = N // TC      # 4
OPC = 512          # output column tile
NOC = HID // OPC   # 4

F32 = mybir.dt.float32
BF16 = mybir.dt.bfloat16
SCALE = float(1.0 / np.sqrt(D))
SWAP_MASK = [p ^ 1 for p in range(32)]  # adjacent-pair swap, uniform per 32-lane group
WARMUP_MMS = 56

_NC = None
LAST_RESULT = None


def _build():
    nc = bacc.Bacc("TRN2", target_bir_lowering=False, debug=False, num_devices=8)

    xT = nc.dram_tensor("xT", [HID, N], BF16, kind="ExternalInput")
    wqk = nc.dram_tensor("wqk", [HID, 2 * QK_COLS], BF16, kind="ExternalInput")
    wv = nc.dram_tensor("wv", [HID, QK_COLS], BF16, kind="ExternalInput")
    wo = nc.dram_tensor("wo", [QK_COLS, HID], BF16, kind="ExternalInput")
    cosT = nc.dram_tensor("cosT", [D, N], BF16, kind="ExternalInput")
    sinT = nc.dram_tensor("sinT", [D, N], BF16, kind="ExternalInput")
    out = nc.dram_tensor("out", [N, HID], BF16, kind="ExternalOutput")

    with tile.TileContext(nc) as tc:
        with tc.tile_pool(name="pqkv", bufs=1) as pqkv:
            qkT = [pqkv.tile([128, N], BF16, name=f"qkT{m}", tag=f"qkT{m}") for m in range(2 * HG)]
            v_sb = [pqkv.tile([128, QK_COLS], BF16, name=f"v{t}", tag=f"v{t}") for t in range(NT)]
            # persistent: the last token chunk's Q-projection is deferred into
            # attention chunk 0 as PE filler; its rope output is first read by
            # chunk-3 attention.
            wqkQ_sb = [
                pqkv.tile([128, 4 * QK_COLS], BF16, name=f"wqkQ{q}", tag=f"wqkQ{q}")
                for q in range(4)
            ]
            xt3 = pqkv.tile([128, KT * TC], BF16, name="xt3", tag="xt3")
            cos_sb = pqkv.tile([D, N], BF16, name="cos_sb", tag="cos")
            sin_sb = pqkv.tile([D, N], BF16, name="sin_sb", tag="sin")

            # ---- stages 1+2: q,k (transposed, roped) and v (natural) ----
            with (
                tc.tile_pool(name="s1w", bufs=1) as s1w,
                tc.tile_pool(name="s1x", bufs=1) as s1x,
                tc.tile_pool(name="s1t", bufs=3) as s1t,
                tc.tile_pool(name="psqk", bufs=8, space="PSUM") as psqk,
            ):
                wqkK_sb = [
                    s1w.tile([128, QK_COLS], BF16, name=f"wqkK{k}", tag=f"wqkK{k}")
                    for k in range(KT)
                ]
                wv_sb = [
                    s1w.tile([128, 8 * QK_COLS], BF16, name=f"wv_sb{hw}", tag=f"wv_sb{hw}")
                    for hw in range(2)
                ]
                xt12 = [
                    s1w.tile([128, KT * TC], BF16, name=f"xt12_{c}", tag=f"xt12_{c}")
                    for c in range(2)
                ]
                scratch = s1w.tile([128, 128], BF16, name="scratch", tag="scratch")

                # PE warm-up: ~56 back-to-back matmuls on a zeroed tile keep
                # the PE busy through the initial DMA wait so the HAM clock
                # gate opens (K=8/8) before the first real matmul.
                nc.vector.memset(scratch[:], 0.0)
                psw = [psqk.tile([128, 128], F32, name="psw", tag="psqk") for _ in range(2)]
                for i in range(WARMUP_MMS):
                    nc.tensor.matmul(
                        psw[i % 2][:, 0:128], scratch[:], scratch[:],
                        start=True, stop=True,
                    )

                # chunk-0 x/wqkK tiles: k-major interleave of (xt[k], wqkK[k])
                # pairs so tile pair k lands before the k-th matmul.  HWDGE
                # (sync+scalar) takes odds, SWDGE (gpsimd) takes even pairs.
                xt0 = [
                    s1x.tile([128, TC], BF16, name=f"xt{k}", tag=f"xt{k}")
                    for k in range(KT)
                ]
                for k in range(KT):
                    if k == 0:
                        nc.sync.dma_start(wqkK_sb[k][:], wqk[k * 128 : (k + 1) * 128, QK_COLS:])
                        nc.scalar.dma_start(xt0[k][:], xT[k * 128 : (k + 1) * 128, 0:TC])
                    elif k % 2 == 0:
                        nc.gpsimd.dma_start(xt0[k][:], xT[k * 128 : (k + 1) * 128, 0:TC])
                        nc.gpsimd.dma_start(wqkK_sb[k][:], wqk[k * 128 : (k + 1) * 128, QK_COLS:])
                    else:
                        nc.scalar.dma_start(xt0[k][:], xT[k * 128 : (k + 1) * 128, 0:TC])
                        nc.sync.dma_start(wqkK_sb[k][:], wqk[k * 128 : (k + 1) * 128, QK_COLS:])

                # wide DMAs for everything else (one descriptor each; the
                # fabric spreads packets over all 16 DMA engines).  Layout of
                # the k-blocked tiles: column block k holds hidden-tile k.
                # wqkQ in 4 paced quarters on the scalar queue (behind the
                # xt0 odds): quarter q lands just before the Q-half consumes
                # hidden-tiles 4q..4q+3 (a single 2MB DMA finished ~12us
                # after the K half was ready)
                for q in range(4):
                    nc.scalar.dma_start(
                        wqkQ_sb[q][:].rearrange("p (k c) -> p k c", k=4),
                        wqk[q * 4 * 128 : (q + 1) * 4 * 128, :QK_COLS].rearrange(
                            "(k p) c -> p k c", k=4
                        ),
                    )
                nc.sync.dma_start(cos_sb[:], cosT[:])
                nc.sync.dma_start(sin_sb[:], sinT[:])
                nc.sync.dma_start(
                    xt12[0][:].rearrange("p (k c) -> p k c", k=KT),
                    xT[:, TC : 2 * TC].rearrange("(k p) c -> p k c", k=KT),
                )
                for hw in range(2):  # wv in halves so chunk-0 V paces
                    nc.gpsimd.dma_start(
                        wv_sb[hw][:].rearrange("p (k c) -> p k c", k=8),
                        wv[hw * 8 * 128 : (hw + 1) * 8 * 128, :].rearrange(
                            "(k p) c -> p k c", k=8
                        ),
                    )
                nc.sync.dma_start(
                    xt12[1][:].rearrange("p (k c) -> p k c", k=KT),
                    xT[:, 2 * TC : 3 * TC].rearrange("(k p) c -> p k c", k=KT),
                )
                nc.scalar.dma_start(
                    xt3[:].rearrange("p (k c) -> p k c", k=KT),
                    xT[:, 3 * TC : 4 * TC].rearrange("(k p) c -> p k c", k=KT),
                )

                def xt_slice(tcn, k):
                    if tcn == 0:
                        return xt0[k][:]
                    if tcn == 3:
                        return xt3[:, k * TC : (k + 1) * TC]
                    return xt12[tcn - 1][:, k * TC : (k + 1) * TC]

                for tcn in range(NTC):
                    tsl = slice(tcn * TC, (tcn + 1) * TC)
                    for half in (1, 0):  # K half first (earlier attention readiness)
                        if half == 0 and tcn == NTC - 1:
                            continue  # last chunk's Q half deferred to stage 3
                        psums = [
                            psqk.tile([128, TC], F32, name="psqk", tag="psqk")
                            for _ in range(4)
                        ]
                        if tcn == 0 and half == 1:
                            # consume HWDGE-delivered odd tiles before the
                            # slower SWDGE evens (accumulation order is free)
                            korder = [0] + list(range(1, KT, 2)) + list(range(2, KT, 2))
                        else:
                            korder = list(range(KT))
                        for ki, k in enumerate(korder):
                            for mi in range(4):
                                if half == 1:
                                    wh = wqkK_sb[k][:, mi * 128 : (mi + 1) * 128]
                                else:
                                    wh = wqkQ_sb[k // 4][:, (k % 4) * QK_COLS + mi * 128 : (k % 4) * QK_COLS + (mi + 1) * 128]
                                nc.tensor.matmul(
                                    psums[mi][:],
                                    wh,
                                    xt_slice(tcn, k),
                                    start=(ki == 0),
                                    stop=(ki == KT - 1),
                                )
                        for mi in range(4):
                            m = half * 4 + mi
                            qsb = s1t.tile([128, TC], BF16, tag="qsb")
                            nc.scalar.activation(qsb[:], psums[mi][:], mybir.ActivationFunctionType.Copy)
                            shuf = s1t.tile([128, TC], BF16, tag="shuf")
                            nc.vector.stream_shuffle(shuf[:], qsb[:], SWAP_MASK)
                            t1 = s1t.tile([128, TC], F32, tag="t1")
                            nc.vector.tensor_tensor(
                                t1[:], qsb[:], cos_sb[:, tsl], mybir.AluOpType.mult
                            )
                            t2 = s1t.tile([128, TC], F32, tag="t2")
                            nc.vector.tensor_tensor(
                                t2[:], shuf[:], sin_sb[:, tsl], mybir.AluOpType.mult
                            )
                            nc.vector.tensor_tensor(
                                qkT[m][:, tsl], t1[:], t2[:], mybir.AluOpType.add
                            )
                    for mtl in range(4):
                        mt = tcn * 4 + mtl
                        pv = psqk.tile([128, QK_COLS], F32, name="psv", tag="psqk")
                        for k in range(KT):
                            nc.tensor.matmul(
                                pv[:],
                                xt_slice(tcn, k)[:, mtl * 128 : (mtl + 1) * 128],
                                wv_sb[k // 8][:, (k % 8) * QK_COLS : (k % 8 + 1) * QK_COLS],
                                start=(k == 0),
                                stop=(k == KT - 1),
                            )
                        if tcn < NTC - 1:
                            # scalar is idle here; keep the DVE free for rope
                            nc.scalar.activation(v_sb[mt][:], pv[:], mybir.ActivationFunctionType.Copy)
                        else:
                            # last chunk: scalar must reach the attention exps
                            # without these queued ahead of them
                            nc.vector.tensor_copy(v_sb[mt][:], pv[:])

            # ---- stages 3-5: attention; out projection of chunk j-1
            # interleaved into the attention loops of chunk j ----
            with (
                tc.tile_pool(name="s3p", bufs=8) as s3p,
                tc.tile_pool(name="s3r", bufs=2) as s3r,
                tc.tile_pool(name="s3s", bufs=4) as s3s,
                tc.tile_pool(name="s3d", bufs=2) as s3d,
                tc.tile_pool(name="s3a", bufs=2) as s3a,
                tc.tile_pool(name="s5w", bufs=1) as s5w,
                tc.tile_pool(name="s5o", bufs=4) as s5o,
                tc.tile_pool(name="pss", bufs=2, space="PSUM") as pss,
                tc.tile_pool(name="pso", bufs=2, space="PSUM") as pso,
                tc.tile_pool(name="psf", bufs=2, space="PSUM") as psf,
            ):
                wo_sb = [s5w.tile([128, HID], BF16, name=f"wo{h}", tag=f"wo{h}") for h in range(HG)]
                for h in range(HG):
                    nc.sync.dma_start(wo_sb[h][:], wo[h * 128 : (h + 1) * 128, :])

                pob_live = {}  # (jq, h) -> normalized bf16 attention tile
                pending_finish = []  # deferred recip+norm emissions; flushed
                # a few iterations into the NEXT head so the reciprocal's
                # wait on the gpsimd all_reduce never head-of-line blocks
                # drains already queued on the DVE FIFO

                def attention_head(jq, h, filler=None):
                    """filler: list of thunks (PE matmul emissions) doled out
                    between the scores and AV matmuls so the PE has work
                    while the scalar engine computes exp."""
                    q0 = jq * TC
                    qsl = slice(q0, q0 + TC)
                    po = pso.tile([128, TC], F32, name="pso", tag="pso")
                    ps_w = s3s.tile([128, 2 * TC], BF16, name="psw", tag="psw")
                    plist = []
                    filler = list(filler) if filler else []
                    for ib in range(NT // 2):
                        ik0, ik1 = 2 * ib, 2 * ib + 1
                        psb = pss.tile([128, 2 * TC], F32, name="psb", tag="psb")
                        nc.tensor.matmul(
                            psb[:, :TC],
                            qkT[HG + h][:, ik0 * 128 : (ik0 + 1) * 128],
                            qkT[h][:, qsl],
                            start=True, stop=True,
                        )
                        nc.tensor.matmul(
                            psb[:, TC:],
                            qkT[HG + h][:, ik1 * 128 : (ik1 + 1) * 128],
                            qkT[h][:, qsl],
                            start=True, stop=True,
                        )
                        p = s3p.tile([128, 2 * TC], BF16, name="p", tag="p")
                        plist.append(p)
                        nc.scalar.activation(
                            p[:], psb[:], mybir.ActivationFunctionType.Exp, scale=SCALE
                        )
                        if filler:
                            filler.pop(0)()
                        nc.tensor.matmul(
                            po[:],
                            v_sb[ik0][:, h * 128 : (h + 1) * 128],
                            p[:, :TC],
                            start=(ib == 0), stop=False,
                        )
                        if filler:
                            filler.pop(0)()
                        nc.tensor.matmul(
                            po[:],
                            v_sb[ik1][:, h * 128 : (h + 1) * 128],
                            p[:, TC:],
                            start=False, stop=(ib == NT // 2 - 1),
                        )
                        # denominator partial sums: one 1024-wide DVE chain
                        if ib == 1:
                            nc.vector.tensor_tensor(ps_w[:], plist[0][:], plist[1][:], mybir.AluOpType.add)
                        elif ib > 1:
                            nc.vector.tensor_tensor(ps_w[:], ps_w[:], p[:], mybir.AluOpType.add)
                        if ib >= 2 and pending_finish:
                            # previous head's all_reduce quarters complete
                            # one-by-one; emit each quarter's recip+norm as
                            # it becomes wait-free (one per iteration)
                            pending_finish.pop(0)()
                    for f in filler:
                        f()
                    return po, ps_w

                QT = TC // 4

                def attention_head_finish(jq, h, po, ps_w):
                    # fold + all_reduce in token-quarters: quarter a covers
                    # tokens a*128:(a+1)*128 = exactly the out-projection
                    # groups with mql==a, so the next head's fillers (all
                    # mql==0) unblock ~2.5us after this head's last exp
                    # instead of waiting the full 512-token reduce chain
                    ps_f = s3s.tile([128, TC], BF16, name="psf2", tag="psf2")
                    den = s3d.tile([128, TC], F32, name="den", tag="den")
                    rden = s3d.tile([128, TC], F32, name="rden", tag="rden")
                    pob = s3a.tile([128, TC], BF16, name=f"pob{h}", tag=f"pob{h}")
                    pob_live[(jq, h)] = pob
                    for a in range(4):
                        sl = slice(a * QT, (a + 1) * QT)
                        slw = slice(TC + a * QT, TC + (a + 1) * QT)
                        nc.vector.tensor_tensor(ps_f[:, sl], ps_w[:, sl], ps_w[:, slw], mybir.AluOpType.add)
                        nc.gpsimd.partition_all_reduce(den[:, sl], ps_f[:, sl], 128, bass_isa.ReduceOp.add)

                    def rest(a):
                        def fn():
                            sl = slice(a * QT, (a + 1) * QT)
                            nc.vector.reciprocal_approx_fast(rden[:, sl], den[:, sl])
                            nc.vector.tensor_tensor(pob[:, sl], po[:, sl], rden[:, sl], mybir.AluOpType.mult)
                        return fn

                    pending_finish.extend(rest(a) for a in range(4))

                def outproj_thunks(jq):
                    """64 PE-matmul thunks for chunk jq's out-projection:
                    16 (token-tile, col-tile) groups, each accumulating over
                    the 4 local head tiles; the last position drains PSUM
                    (DVE, every 4th group on the otherwise-idle scalar) and
                    DMAs the output tile on the sync queue.  The first two
                    groups' h=3 matmuls are deferred ~6 slots so the filler
                    stream doesn't wait on the previous chunk's last-head
                    finish chain (fold -> partition_all_reduce 3.5us ->
                    reciprocal -> normalize, ~6us after its last exp)."""
                    thunks = []
                    for gi, (mql, ft) in enumerate(
                        [(m, f) for m in range(4) for f in range(NOC)]
                    ):
                        mq = jq * 4 + mql
                        holder = {}

                        def mk(h, mql=mql, ft=ft, mq=mq, gi=gi, holder=holder):
                            def thunk():
                                if h == 0:
                                    holder["pf"] = psf.tile([128, OPC], F32, name="psf", tag="psf")
                                pf = holder["pf"]
                                nc.tensor.matmul(
                                    pf[:],
                                    pob_live[(jq, h)][:, mql * 128 : (mql + 1) * 128],
                                    wo_sb[h][:, ft * OPC : (ft + 1) * OPC],
                                    start=(h == 0),
                                    stop=(h == HG - 1),
                                )
                                if h == HG - 1:
                                    ob = s5o.tile([128, OPC], BF16, name="ob", tag="ob")
                                    if gi % 4 == 3:
                                        nc.scalar.activation(
                                            ob[:], pf[:], mybir.ActivationFunctionType.Copy
                                        )
                                    else:
                                        nc.vector.tensor_copy(ob[:], pf[:])
                                    nc.sync.dma_start(
                                        out[mq * 128 : (mq + 1) * 128, ft * OPC : (ft + 1) * OPC],
                                        ob[:],
                                    )
                            return thunk

                        thunks.append([mk(h) for h in range(HG)])
                    # stagger: G0[h0..h2], G1[h0..h2], G0[h3], G1[h3], rest
                    order = thunks[0][:3] + thunks[1][:3] + [thunks[0][3], thunks[1][3]]
                    for g in thunks[2:]:
                        order.extend(g)
                    return order

                def deferred_q_thunks(mi):
                    """The last token chunk's Q projection for head-group
                    column mi: 16 matmuls into a borrowed psf bank, then the
                    PSUM->bf16 copy and rope, writing qkT[mi][:, 1536:2048]
                    (first read by chunk-3 attention)."""
                    tsl = slice((NTC - 1) * TC, NTC * TC)
                    pq = psf.tile([128, TC], F32, name="psq", tag="psf")

                    def mk(k):
                        def thunk():
                            nc.tensor.matmul(
                                pq[:],
                                wqkQ_sb[k // 4][:, (k % 4) * QK_COLS + mi * 128 : (k % 4) * QK_COLS + (mi + 1) * 128],
                                xt3[:, k * TC : (k + 1) * TC],
                                start=(k == 0),
                                stop=(k == KT - 1),
                            )
                            if k == KT - 1:
                                qsb = s3r.tile([128, TC], BF16, tag="qsb")
                                nc.scalar.activation(
                                    qsb[:], pq[:], mybir.ActivationFunctionType.Copy
                                )
                                shuf = s3r.tile([128, TC], BF16, tag="shuf")
                                nc.vector.stream_shuffle(shuf[:], qsb[:], SWAP_MASK)
                                t1 = s3r.tile([128, TC], F32, tag="t1")
                                nc.vector.tensor_tensor(
                                    t1[:], qsb[:], cos_sb[:, tsl], mybir.AluOpType.mult
                                )
                                t2 = s3r.tile([128, TC], F32, tag="t2")
                                nc.vector.tensor_tensor(
                                    t2[:], shuf[:], sin_sb[:, tsl], mybir.AluOpType.mult
                                )
                                nc.vector.tensor_tensor(
                                    qkT[mi][:, tsl], t1[:], t2[:], mybir.AluOpType.add
                                )
                        return thunk

                    return [mk(k) for k in range(KT)]

                for jq in range(NTC):
                    if jq == 0:
                        # cross-assigned (head h builds Q for head h+1) so the
                        # filler never writes the qkT tile the current head's
                        # scores are reading
                        fillers = [t for h in range(HG) for t in deferred_q_thunks((h + 1) % HG)]
                    else:
                        fillers = outproj_thunks(jq - 1)
                    for h in range(HG):
                        po, ps_w = attention_head(
                            jq, h, fillers[h * 16 : (h + 1) * 16]
                        )
                        attention_head_finish(jq, h, po, ps_w)
                # tail: last chunk's projection, back-to-back; the last
                # head's finish quarters interleave ahead of the groups
                # that consume them
                for t_i, t in enumerate(outproj_thunks(NTC - 1)):
                    if t_i % 8 == 0 and pending_finish:
                        pending_finish.pop(0)()
                    t()
                while pending_finish:
                    pending_finish.pop(0)()

    nc.compile()
    return nc


def _get_nc():
    global _NC
    if _NC is None:
        _NC = _build()
    return _NC


def _prep_in_maps(x, rope, qkv_w, out_w):
    x = np.asarray(x, np.float32)
    rope = np.asarray(rope, np.float32)
    qkv_w = np.asarray(qkv_w, np.float32)
    out_w = np.asarray(out_w, np.float32)

    bf = ml_dtypes.bfloat16
    freqs = rope[:, 0, :]  # [N, D]
    cosT = np.ascontiguousarray(np.repeat(freqs[:, 0::2], 2, axis=1).T).astype(bf)
    sinT = np.repeat(freqs[:, 1::2], 2, axis=1).T.copy()
    sinT[0::2, :] *= -1.0  # rope sign folded in: rot[2i] = -q[2i+1]
    sinT = np.ascontiguousarray(sinT).astype(bf)

    qkv3 = qkv_w.reshape(HID, 3, H, D)
    xTs = [np.ascontiguousarray(x[b].T).astype(bf) for b in range(B)]
    in_maps = []
    for core in range(8):
        b, g = core // G, core % G
        hs = slice(g * HG, (g + 1) * HG)
        wq = qkv3[:, 0, hs, :].reshape(HID, QK_COLS)
        wk = qkv3[:, 1, hs, :].reshape(HID, QK_COLS)
        in_maps.append(
            dict(
                xT=xTs[b],
                wqk=np.ascontiguousarray(np.concatenate([wq, wk], axis=1)).astype(bf),
                wv=np.ascontiguousarray(qkv3[:, 2, hs, :].reshape(HID, QK_COLS)).astype(bf),
                wo=np.ascontiguousarray(out_w[g * QK_COLS : (g + 1) * QK_COLS, :]).astype(bf),
                cosT=cosT,
                sinT=sinT,
            )
        )
    return in_maps


def kernel(x, rope, qkv_w, out_w):
    global LAST_RESULT
    nc = _get_nc()
    in_maps = _prep_in_maps(x, rope, qkv_w, out_w)
    res = run_bass_kernel_spmd(nc, in_maps, core_ids=list(range(8)))
    LAST_RESULT = res
    outs = [np.asarray(r["out"], dtype=np.float32) for r in res.results]
    full = np.stack(
        [sum(outs[b * G + g] for g in range(G)) for b in range(B)]
    )
    return full.astype(np.float32)
